# revision 1
# baseline (speedup 1.0000x reference)
"""Trainium2 Bass kernel for multi-head attention decode (B=16, S=8, H=2048,
16 heads x 128 head_dim, KV cache 4096) sharded over 8 NeuronCores by heads
(tensor parallel, 2 heads/core), with an on-device AllReduce after o_proj.

Self-contained: hardcodes all shapes/sharding. Host side only reshapes /
slices / lays out the sharded inputs (incl. storing each core's K-cache
slice pre-transposed as [hd, kv], which is this kernel's preferred on-device
KV-cache layout) and gathers the full output.
"""

import numpy as np

import concourse.bass as bass
import concourse.tile as tile
import concourse.mybir as mybir
from concourse import bacc
from concourse.bass_utils import run_bass_kernel_spmd
from concourse.masks import make_identity

F32 = mybir.dt.float32

N_CORES = 8
B = 16
S = 8
H = 2048
NH = 16           # total heads
HD = 128          # head dim
KV = 4096         # past kv length
NHL = NH // N_CORES   # heads per core = 2
TOK = B * S       # 128 tokens
KT_GROUP = 1024   # kv elements per DMA tile
N_GROUPS = KV // KT_GROUP   # 4
CPG = KT_GROUP // 128       # chunks of 128 kv per group = 8
SCALE = 1.0 / float(np.sqrt(HD))

_CACHED_NC = None


def _build_nc(with_collective=True, debug_taps=False, repeat=None,
              ablate=frozenset(), kv_bufs=2, kv_split=1):
    """repeat=R builds a benchmark variant: whole body in a For_i loop R times,
    no collective (collectives can't live in control flow).
    ablate: subset of {"noexp", "noattnv", "nors"} for perf bisection
    (outputs are wrong for any non-empty ablate)."""
    ablate = frozenset(ablate)
    if repeat is not None:
        with_collective = False
    nc = bacc.Bacc(
        "TRN2",
        target_bir_lowering=False,
        debug=False,
        enable_asserts=False,
        num_devices=N_CORES if with_collective else 1,
    )

    kt = nc.dram_tensor("kt", [NHL, B, HD, KV], F32, kind="ExternalInput")
    vc = nc.dram_tensor("vc", [B, NHL, KV, HD], F32, kind="ExternalInput")
    xt = nc.dram_tensor("xt", [H, TOK], F32, kind="ExternalInput")
    wq = nc.dram_tensor("wq", [H, NHL * HD], F32, kind="ExternalInput")
    wk = nc.dram_tensor("wk", [H, NHL * HD], F32, kind="ExternalInput")
    wv = nc.dram_tensor("wv", [H, NHL * HD], F32, kind="ExternalInput")
    wo = nc.dram_tensor("wo", [NHL * HD, H], F32, kind="ExternalInput")
    cost = nc.dram_tensor("cost", [HD, TOK], F32, kind="ExternalInput")
    sint = nc.dram_tensor("sint", [HD, TOK], F32, kind="ExternalInput")
    prot = nc.dram_tensor("prot", [HD, HD], F32, kind="ExternalInput")
    out = nc.dram_tensor("out", [TOK, H], F32, kind="ExternalOutput")

    KT16 = H // 128  # 16 contraction tiles over H

    with tile.TileContext(nc) as tc:
        with tc.tile_pool(name="const", bufs=1) as const:
            # Resident weights / activations
            xt_sb = const.tile([128, KT16, TOK], F32)
            nc.sync.dma_start(out=xt_sb, in_=xt.ap().rearrange("(t p) n -> p t n", p=128))
            wq_sb = const.tile([128, KT16, NHL * HD], F32)
            nc.sync.dma_start(out=wq_sb, in_=wq.ap().rearrange("(t p) m -> p t m", p=128))
            wk_sb = const.tile([128, KT16, NHL * HD], F32)
            nc.sync.dma_start(out=wk_sb, in_=wk.ap().rearrange("(t p) m -> p t m", p=128))
            wv_sb = const.tile([128, KT16, NHL * HD], F32)
            nc.sync.dma_start(out=wv_sb, in_=wv.ap().rearrange("(t p) m -> p t m", p=128))
            wo_sb = const.tile([128, NHL, H], F32)
            nc.sync.dma_start(out=wo_sb, in_=wo.ap().rearrange("(t p) n -> p t n", p=128))
            cost_sb = const.tile([HD, TOK], F32)
            nc.sync.dma_start(out=cost_sb, in_=cost.ap())
            sint_sb = const.tile([HD, TOK], F32)
            nc.sync.dma_start(out=sint_sb, in_=sint.ap())
            prot_sb = const.tile([HD, HD], F32)
            nc.sync.dma_start(out=prot_sb, in_=prot.ap())
            ones_sb = const.tile([128, 1], F32)
            nc.vector.memset(ones_sb, 1.0)
            id8_sb = const.tile([8, 8], F32)
            make_identity(nc, id8_sb)

            onesrow_sb = const.tile([1, TOK], F32)
            nc.vector.memset(onesrow_sb, 1.0)
            qT = [const.tile([HD, TOK], F32, name=f"qT{h}") for h in range(NHL)]
            kT = [const.tile([HD, TOK], F32, name=f"kT{h}") for h in range(NHL)]
            vstage = [const.tile([S, B, HD], F32, name=f"vstage{h}") for h in range(NHL)]
            # unnormalized attention out (transposed) + per-token softmax denoms
            oTu_sb = [const.tile([HD, TOK], F32, name=f"oTu{h}") for h in range(NHL)]
            rsh_sb = [const.tile([1, TOK], F32, name=f"rsh{h}") for h in range(NHL)]
            oT_sb = [const.tile([HD, TOK], F32, name=f"oT{h}") for h in range(NHL)]

            _loop = None
            if repeat is not None:
                _loop = tc.For_i(0, repeat, 1)
                _loop.__enter__()

            # ---- Phase 1: projections + RoPE (all in [hd, tok] layout) ----
            with tc.tile_pool(name="proj_ps", bufs=2, space="PSUM") as pps, \
                 tc.tile_pool(name="proj_tmp", bufs=2) as ptp:
                for h in range(NHL):
                    for w_sb, dst in ((wq_sb, qT[h]), (wk_sb, kT[h])):
                        ps = pps.tile([128, 128], F32, tag="projps", name="ps")
                        for t in range(KT16):
                            nc.tensor.matmul(
                                ps,
                                lhsT=w_sb[:, t, h * HD:(h + 1) * HD],
                                rhs=xt_sb[:, t, :],
                                start=(t == 0),
                                stop=(t == KT16 - 1),
                            )
                        raw = ptp.tile([128, 128], F32, tag="raw", name="raw")
                        nc.vector.tensor_copy(out=raw, in_=ps)
                        rot_ps = pps.tile([128, 128], F32, tag="projps", name="rot_ps")
                        nc.tensor.matmul(rot_ps, lhsT=prot_sb, rhs=raw,
                                         start=True, stop=True)
                        tmp = ptp.tile([128, 128], F32, tag="tmp", name="tmp")
                        nc.vector.tensor_mul(out=tmp, in0=raw, in1=cost_sb)
                        nc.vector.tensor_mul(out=dst, in0=rot_ps, in1=sint_sb)
                        nc.vector.tensor_add(out=dst, in0=dst, in1=tmp)

                # v_new = x @ Wv  -> [tok, 2*128] (natural layout)
                ps_v = pps.tile([128, NHL * HD], F32, tag="projps", name="ps_v")
                for t in range(KT16):
                    nc.tensor.matmul(ps_v, lhsT=xt_sb[:, t, :], rhs=wv_sb[:, t, :],
                                     start=(t == 0), stop=(t == KT16 - 1))
                vnew_sb = ptp.tile([128, NHL * HD], F32, tag="vnew", name="vnew_sb")
                nc.vector.tensor_copy(out=vnew_sb, in_=ps_v)
                # restage per (head, batch) at partition base 0: [s, b, hd]
                for h in range(NHL):
                    for b in range(B):
                        nc.gpsimd.dma_start(
                            out=vstage[h][:, b, :],
                            in_=vnew_sb[b * S:(b + 1) * S, h * HD:(h + 1) * HD],
                        )

            # ---- Phase 2: attention over the KV cache ----
            # Per (h,b): one 2MB kt DMA + one ~2MB vt DMA; 32 scores matmuls
            # into one PSUM bank; ONE exp; 33 attn@V matmuls accumulating
            # o+rowsum (ones column fused into V tiles). Software-pipelined by
            # one (h,b) step: scores(i) are emitted before attn@V(i-1) so PE
            # stays busy while ACT runs exp(i-1).
            if ablate:
                # keep downstream consumers NaN-free
                for h in range(NHL):
                    nc.vector.memset(oTu_sb[h], 1.0)
                    nc.vector.memset(rsh_sb[h], 1.0)

            NCH = KV // 128  # 32 chunks per (h,b)
            with tc.tile_pool(name="kv_io", bufs=kv_bufs) as kvp, \
                 tc.tile_pool(name="esb", bufs=2) as etp, \
                 tc.tile_pool(name="ps_s", bufs=2, space="PSUM") as psp, \
                 tc.tile_pool(name="ps_o", bufs=2, space="PSUM") as pso, \
                 tc.tile_pool(name="ps_rs", bufs=2, space="PSUM") as psr:
                hb = [(h, b) for h in range(NHL) for b in range(B)]

                stage = {}  # pipelined state for step i

                def emit_scores(i):
                    h, b = hb[i]
                    qcol = qT[h][:, b * S:(b + 1) * S]
                    kt_t = kvp.tile([128, KV], F32, tag="kt", name="kt_t")
                    v_t = kvp.tile([128, NCH, 128], F32, tag="vt", name="v_t")
                    KSP = KV // kv_split
                    CSP = NCH // kv_split
                    for sp in range(kv_split):
                        nc.sync.dma_start(
                            out=kt_t[:, sp * KSP:(sp + 1) * KSP],
                            in_=kt.ap()[h, b, :, sp * KSP:(sp + 1) * KSP])
                        nc.scalar.dma_start(
                            out=v_t[:, sp * CSP:(sp + 1) * CSP, :],
                            in_=vc.ap()[b, h, sp * KSP:(sp + 1) * KSP, :]
                                .rearrange("(c p) d -> p c d", p=128))
                    # cols 0..255: past-kv scores; cols 256..263: new-token scores
                    s_ps = psp.tile([128, (NCH + 1) * S], F32, tag="sps", name="s_ps")
                    for c in range(NCH):
                        nc.tensor.matmul(
                            s_ps[:, c * S:(c + 1) * S],
                            lhsT=kt_t[:, c * 128:(c + 1) * 128],
                            rhs=qcol,
                            start=True, stop=True)
                    nc.tensor.matmul(
                        s_ps[0:S, NCH * S:(NCH + 1) * S],
                        lhsT=kT[h][:, b * S:(b + 1) * S],
                        rhs=qcol, start=True, stop=True)
                    if "noexp" in ablate:
                        stage[i] = (None, v_t)
                        return
                    eT = etp.tile([128, (NCH + 1) * S], F32, tag="eT", name="eT")
                    nc.scalar.activation(out=eT[:, 0:NCH * S], in_=s_ps[:, 0:NCH * S],
                                         func=mybir.ActivationFunctionType.Exp,
                                         scale=SCALE)
                    nc.scalar.activation(out=eT[0:S, NCH * S:(NCH + 1) * S],
                                         in_=s_ps[0:S, NCH * S:(NCH + 1) * S],
                                         func=mybir.ActivationFunctionType.Exp,
                                         scale=SCALE)
                    stage[i] = (eT, v_t)

                def emit_attnv(i):
                    h, b = hb[i]
                    qcol = qT[h][:, b * S:(b + 1) * S]
                    eT, v_t = stage.pop(i)
                    if eT is None or "noattnv" in ablate:
                        return
                    eTn = eT[0:S, NCH * S:(NCH + 1) * S]
                    # oT2_ps[d, slot, s]: rotating accumulators in SEPARATE
                    # PSUM banks (bank = 512 f32) so consecutive matmuls never
                    # RMW the same accumulation address (drain pipelining)
                    NSLOT = 2
                    oT2_ps = pso.tile([HD, NSLOT, 512], F32, tag="ops", name="oT2_ps")
                    for c in range(NCH):
                        nc.tensor.matmul(
                            oT2_ps[:, c % NSLOT, 0:S],
                            lhsT=v_t[:, c, :],
                            rhs=eT[:, c * S:(c + 1) * S],
                            start=(c < NSLOT),
                            stop=(c >= NCH - NSLOT + 1))
                    # new tokens (kv positions 4096..4103) -> slot 0, last
                    nc.tensor.matmul(oT2_ps[:, 0, 0:S], lhsT=vstage[h][:, b, :],
                                     rhs=eTn, start=False, stop=True)
                    # rowsums: ones^T @ eT -> [1, (c s)] partials in one matmul
                    do_rs = "nors" not in ablate
                    if do_rs:
                        rs_ps = psr.tile([1, (NCH + 1) * S], F32, tag="rsps",
                                         name="rs_ps")
                        nc.tensor.matmul(rs_ps[:, 0:NCH * S], lhsT=ones_sb,
                                         rhs=eT[:, 0:NCH * S],
                                         start=True, stop=False)
                        nc.tensor.matmul(rs_ps[:, NCH * S:(NCH + 1) * S],
                                         lhsT=ones_sb[0:S, :],
                                         rhs=eTn, start=False, stop=True)
                    # evacuate: fold the slots -> unnormalized oT column block
                    nc.vector.reduce_sum(
                        out=oTu_sb[h][:, b * S:(b + 1) * S],
                        in_=oT2_ps[:, :, 0:S].rearrange("p g s -> p s g"),
                        axis=mybir.AxisListType.X)
                    if do_rs:
                        nc.vector.reduce_sum(
                            out=rsh_sb[h][:, b * S:(b + 1) * S],
                            in_=rs_ps.rearrange("p (c s) -> p s c", s=S),
                            axis=mybir.AxisListType.X)

                emit_scores(0)
                for i in range(1, len(hb)):
                    emit_scores(i)
                    emit_attnv(i - 1)
                emit_attnv(len(hb) - 1)

            # ---- Phase 3: normalize per head: oT = oTu * (1/rs) broadcast ----
            with tc.tile_pool(name="ps_bc", bufs=2, space="PSUM") as pbc, \
                 tc.tile_pool(name="nrm", bufs=2) as nrm:
                for h in range(NHL):
                    recip = nrm.tile([1, TOK], F32, tag="recip", name="recip")
                    nc.vector.reciprocal(out=recip, in_=rsh_sb[h])
                    bc_ps = pbc.tile([HD, TOK], F32, tag="bc", name="bc_ps")
                    nc.tensor.matmul(bc_ps, lhsT=onesrow_sb, rhs=recip,
                                     start=True, stop=True)
                    nc.vector.tensor_mul(out=oT_sb[h], in0=oTu_sb[h], in1=bc_ps)

            # ---- Phase 4: o_proj + AllReduce ----
            with tc.tile_pool(name="ps_y", bufs=2, space="PSUM") as psy, \
                 tc.tile_pool(name="ysb", bufs=1) as yp, \
                 tc.tile_pool(name="dram", bufs=1, space="DRAM") as dram:
                y_sb = yp.tile([TOK, H], F32, name="y_sb")
                for nb in range(H // 512):
                    y_ps = psy.tile([TOK, 512], F32, tag="yps", name="y_ps")
                    for h in range(NHL):
                        nc.tensor.matmul(
                            y_ps,
                            lhsT=oT_sb[h],
                            rhs=wo_sb[:, h, nb * 512:(nb + 1) * 512],
                            start=(h == 0), stop=(h == NHL - 1))
                    nc.vector.tensor_copy(out=y_sb[:, nb * 512:(nb + 1) * 512], in_=y_ps)
                if with_collective:
                    y_in = dram.tile([TOK, H], F32, name="y_in")
                    nc.sync.dma_start(out=y_in, in_=y_sb)
                    y_out = dram.tile([TOK, H], F32, addr_space="Shared", name="y_out")
                    nc.gpsimd.collective_compute(
                        "AllReduce",
                        mybir.AluOpType.add,
                        replica_groups=[list(range(N_CORES))],
                        ins=[y_in[:]],
                        outs=[y_out[:]],
                    )
                    nc.sync.dma_start(out=out.ap(), in_=y_out)
                else:
                    nc.sync.dma_start(out=out.ap(), in_=y_sb)

            if _loop is not None:
                _loop.__exit__(None, None, None)

            if debug_taps:
                for h in range(NHL):
                    for nm, sb in (
                        (f"dbg_qT{h}", qT[h]),
                        (f"dbg_kT{h}", kT[h]),
                        (f"dbg_oT{h}", oT_sb[h]),
                    ):
                        d = nc.dram_tensor(nm, list(sb.shape), F32, kind="ExternalOutput")
                        nc.sync.dma_start(out=d.ap(), in_=sb)
                    dv = nc.dram_tensor(f"dbg_vst{h}", [S, B, HD], F32, kind="ExternalOutput")
                    nc.sync.dma_start(out=dv.ap(), in_=vstage[h])

    nc.compile()
    return nc


def get_nc():
    global _CACHED_NC
    if _CACHED_NC is None:
        _CACHED_NC = _build_nc()
    return _CACHED_NC


def _rope_tables():
    inv_freq = (1.0 / (10000.0 ** (np.arange(0, HD, 2, dtype=np.float32) / HD))).astype(np.float32)
    t = np.arange(S, dtype=np.float32)
    freqs = t[:, None] * inv_freq[None, :]          # [S, HD/2]
    emb = np.concatenate([freqs, freqs], axis=-1)   # [S, HD]
    cos = np.cos(emb).astype(np.float32)            # [S, HD]
    sin = np.sin(emb).astype(np.float32)
    # transposed+tiled over batches: [HD, B*S] with col b*S+s = table row s
    cosT = np.tile(cos.T, (1, B)).astype(np.float32)
    sinT = np.tile(sin.T, (1, B)).astype(np.float32)
    return np.ascontiguousarray(cosT), np.ascontiguousarray(sinT)


def _rot_matrix():
    # rot(q)[d] = -q[d+64] (d<64) ; q[d-64] (d>=64);  rot = P @ q (q as [hd] col)
    P = np.zeros((HD, HD), dtype=np.float32)
    half = HD // 2
    for d in range(half):
        P[d, d + half] = -1.0
        P[d + half, d] = 1.0
    return np.ascontiguousarray(P.T)  # lhsT for out = P @ rhs


def make_in_maps(x, Wq, Wk, Wv, Wo, past_k, past_v):
    xt = np.ascontiguousarray(x.reshape(TOK, H).T)
    cosT, sinT = _rope_tables()
    prot = _rot_matrix()
    in_maps = []
    for c in range(N_CORES):
        h0 = c * NHL
        cols = slice(h0 * HD, (h0 + NHL) * HD)
        in_maps.append({
            "kt": np.ascontiguousarray(
                past_k[:, h0:h0 + NHL].transpose(1, 0, 3, 2)),   # [nhl, B, HD, KV]
            "vc": np.ascontiguousarray(past_v[:, h0:h0 + NHL]),    # [B, nhl, KV, HD]
            "xt": xt,
            "wq": np.ascontiguousarray(Wq[:, cols]),
            "wk": np.ascontiguousarray(Wk[:, cols]),
            "wv": np.ascontiguousarray(Wv[:, cols]),
            "wo": np.ascontiguousarray(Wo[cols, :]),
            "cost": cosT,
            "sint": sinT,
            "prot": prot,
        })
    return in_maps


def kernel(x, Wq, Wk, Wv, Wo, past_k, past_v):
    x = np.asarray(x, dtype=np.float32)
    Wq = np.asarray(Wq, dtype=np.float32)
    Wk = np.asarray(Wk, dtype=np.float32)
    Wv = np.asarray(Wv, dtype=np.float32)
    Wo = np.asarray(Wo, dtype=np.float32)
    past_k = np.asarray(past_k, dtype=np.float32)
    past_v = np.asarray(past_v, dtype=np.float32)

    nc = get_nc()
    in_maps = make_in_maps(x, Wq, Wk, Wv, Wo, past_k, past_v)
    res = run_bass_kernel_spmd(nc, in_maps, core_ids=list(range(N_CORES)))
    y = res.results[0]["out"]
    return np.asarray(y, dtype=np.float32).reshape(B, S, H)



# revision 2
# speedup vs baseline: 3.1619x; 3.1619x over previous
"""Trainium2 Bass kernel for multi-head attention decode (B=16, S=8, H=2048,
16 heads x 128 head_dim, KV cache 4096) sharded over 8 NeuronCores by heads
(tensor parallel, 2 heads/core), with an on-device AllReduce after o_proj.

Compute/storage dtype is bf16 (KV cache, weights, activations; all matmul
accumulation in fp32 PSUM) which halves HBM traffic vs f32 and enables the
PE fast-weight-load path. The K cache is stored pre-transposed [hd, kv] and
the V cache chunk-transposed [kv%128, kv//128, hd] on host so both stream
as fully contiguous 8KB-per-partition DMAs.

Self-contained: hardcodes all shapes/sharding. Host side only reshapes /
slices / casts the sharded inputs and gathers the full output.
"""

import numpy as np
import ml_dtypes

import concourse.bass as bass
import concourse.tile as tile
import concourse.mybir as mybir
from concourse import bacc
from concourse.bass_utils import run_bass_kernel_spmd

F32 = mybir.dt.float32
BF16 = mybir.dt.bfloat16
NP_BF16 = ml_dtypes.bfloat16

N_CORES = 8
B = 16
S = 8
H = 2048
NH = 16           # total heads
HD = 128          # head dim
KV = 4096         # past kv length
NHL = NH // N_CORES   # heads per core = 2
TOK = B * S       # 128 tokens
NCH = KV // 128   # 32 kv chunks of 128 per (h,b)
SCALE = 1.0 / float(np.sqrt(HD))

_CACHED_NC = None


def _build_nc(with_collective=True, repeat=None, kv_bufs=3):
    """repeat=R builds a benchmark variant: whole body in a For_i loop R times,
    no collective (collectives can't live in control flow)."""
    if repeat is not None:
        with_collective = False
    nc = bacc.Bacc(
        "TRN2",
        target_bir_lowering=False,
        debug=False,
        enable_asserts=False,
        num_devices=N_CORES if with_collective else 1,
    )

    kt = nc.dram_tensor("kt", [NHL, B, HD, KV], BF16, kind="ExternalInput")
    vc = nc.dram_tensor("vc", [NHL, B, 128, NCH, 128], BF16, kind="ExternalInput")
    xt = nc.dram_tensor("xt", [H, TOK], BF16, kind="ExternalInput")
    wq = nc.dram_tensor("wq", [H, NHL * HD], BF16, kind="ExternalInput")
    wk = nc.dram_tensor("wk", [H, NHL * HD], BF16, kind="ExternalInput")
    wv = nc.dram_tensor("wv", [H, NHL * HD], BF16, kind="ExternalInput")
    wo = nc.dram_tensor("wo", [NHL * HD, H], BF16, kind="ExternalInput")
    cost = nc.dram_tensor("cost", [HD, TOK], BF16, kind="ExternalInput")
    sint = nc.dram_tensor("sint", [HD, TOK], BF16, kind="ExternalInput")
    prot = nc.dram_tensor("prot", [HD, HD], BF16, kind="ExternalInput")
    out = nc.dram_tensor("out", [TOK, H], F32, kind="ExternalOutput")

    KT16 = H // 128  # 16 contraction tiles over H

    with tile.TileContext(nc) as tc:
        with tc.tile_pool(name="const", bufs=1) as const:
            # Resident weights / activations
            xt_sb = const.tile([128, KT16, TOK], BF16)
            nc.sync.dma_start(out=xt_sb, in_=xt.ap().rearrange("(t p) n -> p t n", p=128))
            wq_sb = const.tile([128, KT16, NHL * HD], BF16)
            nc.sync.dma_start(out=wq_sb, in_=wq.ap().rearrange("(t p) m -> p t m", p=128))
            wk_sb = const.tile([128, KT16, NHL * HD], BF16)
            nc.sync.dma_start(out=wk_sb, in_=wk.ap().rearrange("(t p) m -> p t m", p=128))
            wv_sb = const.tile([128, KT16, NHL * HD], BF16)
            nc.sync.dma_start(out=wv_sb, in_=wv.ap().rearrange("(t p) m -> p t m", p=128))
            wo_sb = const.tile([128, NHL, H], BF16)
            nc.sync.dma_start(out=wo_sb, in_=wo.ap().rearrange("(t p) n -> p t n", p=128))
            cost_sb = const.tile([HD, TOK], BF16)
            nc.sync.dma_start(out=cost_sb, in_=cost.ap())
            sint_sb = const.tile([HD, TOK], BF16)
            nc.sync.dma_start(out=sint_sb, in_=sint.ap())
            prot_sb = const.tile([HD, HD], BF16)
            nc.sync.dma_start(out=prot_sb, in_=prot.ap())
            ones_sb = const.tile([128, 1], BF16)
            nc.vector.memset(ones_sb, 1.0)

            onesrow_sb = const.tile([1, TOK], F32)
            nc.vector.memset(onesrow_sb, 1.0)
            qT = [const.tile([HD, TOK], BF16, name=f"qT{h}") for h in range(NHL)]
            kT = [const.tile([HD, TOK], BF16, name=f"kT{h}") for h in range(NHL)]
            vstage = [const.tile([S, B, HD], BF16, name=f"vstage{h}") for h in range(NHL)]
            # unnormalized attention out (transposed) + per-token softmax denoms
            oTu_sb = [const.tile([HD, TOK], F32, name=f"oTu{h}") for h in range(NHL)]
            rsh_sb = [const.tile([1, TOK], F32, name=f"rsh{h}") for h in range(NHL)]
            oT_sb = [const.tile([HD, TOK], BF16, name=f"oT{h}") for h in range(NHL)]

            _loop = None
            if repeat is not None:
                _loop = tc.For_i(0, repeat, 1)
                _loop.__enter__()

            # ---- Phase 1: projections + RoPE (all in [hd, tok] layout) ----
            with tc.tile_pool(name="proj_ps", bufs=2, space="PSUM") as pps, \
                 tc.tile_pool(name="proj_tmp", bufs=2) as ptp:
                for h in range(NHL):
                    for w_sb, dst in ((wq_sb, qT[h]), (wk_sb, kT[h])):
                        ps = pps.tile([128, 128], F32, tag="projps", name="ps")
                        for t in range(KT16):
                            nc.tensor.matmul(
                                ps,
                                lhsT=w_sb[:, t, h * HD:(h + 1) * HD],
                                rhs=xt_sb[:, t, :],
                                start=(t == 0),
                                stop=(t == KT16 - 1),
                            )
                        raw = ptp.tile([128, 128], BF16, tag="raw", name="raw")
                        nc.vector.tensor_copy(out=raw, in_=ps)
                        rot_ps = pps.tile([128, 128], F32, tag="projps", name="rot_ps")
                        nc.tensor.matmul(rot_ps, lhsT=prot_sb, rhs=raw,
                                         start=True, stop=True)
                        tmp = ptp.tile([128, 128], F32, tag="tmp", name="tmp")
                        nc.vector.tensor_mul(out=tmp, in0=raw, in1=cost_sb)
                        tmp2 = ptp.tile([128, 128], F32, tag="tmp2", name="tmp2")
                        nc.vector.tensor_mul(out=tmp2, in0=rot_ps, in1=sint_sb)
                        nc.vector.tensor_add(out=dst, in0=tmp2, in1=tmp)

                # v_new = x @ Wv  -> [tok, 2*128] (natural layout)
                ps_v = pps.tile([128, NHL * HD], F32, tag="projps", name="ps_v")
                for t in range(KT16):
                    nc.tensor.matmul(ps_v, lhsT=xt_sb[:, t, :], rhs=wv_sb[:, t, :],
                                     start=(t == 0), stop=(t == KT16 - 1))
                vnew_sb = ptp.tile([128, NHL * HD], BF16, tag="vnew", name="vnew_sb")
                nc.vector.tensor_copy(out=vnew_sb, in_=ps_v)
                # restage per (head, batch) at partition base 0: [s, b, hd]
                for h in range(NHL):
                    for b in range(B):
                        nc.gpsimd.dma_start(
                            out=vstage[h][:, b, :],
                            in_=vnew_sb[b * S:(b + 1) * S, h * HD:(h + 1) * HD],
                        )

            # ---- Phase 2: attention over the KV cache ----
            # Per (h,b): one 1MB kt DMA + one 1MB vt DMA (both fully
            # contiguous); 32 scores matmuls into one PSUM bank; ONE exp;
            # 33 attn@V matmuls + rowsum matmuls. Software-pipelined by
            # one (h,b) step: scores(i) are emitted before attn@V(i-1) so PE
            # stays busy while ACT runs exp(i-1).
            with tc.tile_pool(name="kv_io", bufs=kv_bufs) as kvp, \
                 tc.tile_pool(name="esb", bufs=2) as etp, \
                 tc.tile_pool(name="ps_s", bufs=2, space="PSUM") as psp, \
                 tc.tile_pool(name="ps_o", bufs=2, space="PSUM") as pso, \
                 tc.tile_pool(name="ps_rs", bufs=2, space="PSUM") as psr:
                hb = [(h, b) for h in range(NHL) for b in range(B)]

                stage = {}  # pipelined state for step i

                def emit_scores(i):
                    h, b = hb[i]
                    qcol = qT[h][:, b * S:(b + 1) * S]
                    kt_t = kvp.tile([128, KV], BF16, tag="kt", name="kt_t")
                    v_t = kvp.tile([128, NCH, 128], BF16, tag="vt", name="v_t")
                    nc.sync.dma_start(out=kt_t, in_=kt.ap()[h, b])
                    nc.scalar.dma_start(out=v_t, in_=vc.ap()[h, b])
                    # cols 0..255: past-kv scores; cols 256..263: new-token scores
                    s_ps = psp.tile([128, (NCH + 1) * S], F32, tag="sps", name="s_ps")
                    for c in range(NCH):
                        nc.tensor.matmul(
                            s_ps[:, c * S:(c + 1) * S],
                            lhsT=kt_t[:, c * 128:(c + 1) * 128],
                            rhs=qcol,
                            start=True, stop=True)
                    nc.tensor.matmul(
                        s_ps[0:S, NCH * S:(NCH + 1) * S],
                        lhsT=kT[h][:, b * S:(b + 1) * S],
                        rhs=qcol, start=True, stop=True)
                    eT = etp.tile([128, (NCH + 1) * S], BF16, tag="eT", name="eT")
                    nc.scalar.activation(out=eT[:, 0:NCH * S], in_=s_ps[:, 0:NCH * S],
                                         func=mybir.ActivationFunctionType.Exp,
                                         scale=SCALE)
                    nc.scalar.activation(out=eT[0:S, NCH * S:(NCH + 1) * S],
                                         in_=s_ps[0:S, NCH * S:(NCH + 1) * S],
                                         func=mybir.ActivationFunctionType.Exp,
                                         scale=SCALE)
                    stage[i] = (eT, v_t)

                def emit_attnv(i):
                    h, b = hb[i]
                    eT, v_t = stage.pop(i)
                    eTn = eT[0:S, NCH * S:(NCH + 1) * S]
                    # oT2_ps[d, slot, s]: rotating accumulators in SEPARATE
                    # PSUM banks (bank = 512 f32) so consecutive matmuls never
                    # RMW the same accumulation address (drain pipelining)
                    NSLOT = 2
                    oT2_ps = pso.tile([HD, NSLOT, 512], F32, tag="ops", name="oT2_ps")
                    for c in range(NCH):
                        nc.tensor.matmul(
                            oT2_ps[:, c % NSLOT, 0:S],
                            lhsT=v_t[:, c, :],
                            rhs=eT[:, c * S:(c + 1) * S],
                            start=(c < NSLOT),
                            stop=(c >= NCH - NSLOT + 1))
                    # new tokens (kv positions 4096..4103) -> slot 0, last
                    nc.tensor.matmul(oT2_ps[:, 0, 0:S], lhsT=vstage[h][:, b, :],
                                     rhs=eTn, start=False, stop=True)
                    # rowsums: ones^T @ eT -> [1, (c s)] partials in one matmul
                    rs_ps = psr.tile([1, (NCH + 1) * S], F32, tag="rsps",
                                     name="rs_ps")
                    nc.tensor.matmul(rs_ps[:, 0:NCH * S], lhsT=ones_sb,
                                     rhs=eT[:, 0:NCH * S],
                                     start=True, stop=False)
                    nc.tensor.matmul(rs_ps[:, NCH * S:(NCH + 1) * S],
                                     lhsT=ones_sb[0:S, :],
                                     rhs=eTn, start=False, stop=True)
                    # evacuate: fold the slots -> unnormalized oT column block
                    nc.vector.reduce_sum(
                        out=oTu_sb[h][:, b * S:(b + 1) * S],
                        in_=oT2_ps[:, :, 0:S].rearrange("p g s -> p s g"),
                        axis=mybir.AxisListType.X)
                    nc.vector.reduce_sum(
                        out=rsh_sb[h][:, b * S:(b + 1) * S],
                        in_=rs_ps.rearrange("p (c s) -> p s c", s=S),
                        axis=mybir.AxisListType.X)

                emit_scores(0)
                for i in range(1, len(hb)):
                    emit_scores(i)
                    emit_attnv(i - 1)
                emit_attnv(len(hb) - 1)

            # ---- Phase 3: normalize per head: oT = oTu * (1/rs) broadcast ----
            with tc.tile_pool(name="ps_bc", bufs=2, space="PSUM") as pbc, \
                 tc.tile_pool(name="nrm", bufs=2) as nrm:
                for h in range(NHL):
                    recip = nrm.tile([1, TOK], F32, tag="recip", name="recip")
                    nc.vector.reciprocal(out=recip, in_=rsh_sb[h])
                    bc_ps = pbc.tile([HD, TOK], F32, tag="bc", name="bc_ps")
                    nc.tensor.matmul(bc_ps, lhsT=onesrow_sb, rhs=recip,
                                     start=True, stop=True)
                    nc.vector.tensor_mul(out=oT_sb[h], in0=oTu_sb[h], in1=bc_ps)

            # ---- Phase 4: o_proj + AllReduce ----
            with tc.tile_pool(name="ps_y", bufs=2, space="PSUM") as psy, \
                 tc.tile_pool(name="ysb", bufs=1) as yp, \
                 tc.tile_pool(name="dram", bufs=1, space="DRAM") as dram:
                y_sb = yp.tile([TOK, H], F32, name="y_sb")
                for nb in range(H // 512):
                    y_ps = psy.tile([TOK, 512], F32, tag="yps", name="y_ps")
                    for h in range(NHL):
                        nc.tensor.matmul(
                            y_ps,
                            lhsT=oT_sb[h],
                            rhs=wo_sb[:, h, nb * 512:(nb + 1) * 512],
                            start=(h == 0), stop=(h == NHL - 1))
                    nc.vector.tensor_copy(out=y_sb[:, nb * 512:(nb + 1) * 512], in_=y_ps)
                if with_collective:
                    y_in = dram.tile([TOK, H], F32, name="y_in")
                    nc.sync.dma_start(out=y_in, in_=y_sb)
                    y_out = dram.tile([TOK, H], F32, addr_space="Shared", name="y_out")
                    nc.gpsimd.collective_compute(
                        "AllReduce",
                        mybir.AluOpType.add,
                        replica_groups=[list(range(N_CORES))],
                        ins=[y_in[:]],
                        outs=[y_out[:]],
                    )
                    nc.sync.dma_start(out=out.ap(), in_=y_out)
                else:
                    nc.sync.dma_start(out=out.ap(), in_=y_sb)

            if _loop is not None:
                _loop.__exit__(None, None, None)

    nc.compile()
    return nc


def get_nc():
    global _CACHED_NC
    if _CACHED_NC is None:
        _CACHED_NC = _build_nc()
    return _CACHED_NC


def _rope_tables():
    inv_freq = (1.0 / (10000.0 ** (np.arange(0, HD, 2, dtype=np.float32) / HD))).astype(np.float32)
    t = np.arange(S, dtype=np.float32)
    freqs = t[:, None] * inv_freq[None, :]          # [S, HD/2]
    emb = np.concatenate([freqs, freqs], axis=-1)   # [S, HD]
    cos = np.cos(emb).astype(np.float32)            # [S, HD]
    sin = np.sin(emb).astype(np.float32)
    # transposed+tiled over batches: [HD, B*S] with col b*S+s = table row s
    cosT = np.tile(cos.T, (1, B)).astype(NP_BF16)
    sinT = np.tile(sin.T, (1, B)).astype(NP_BF16)
    return np.ascontiguousarray(cosT), np.ascontiguousarray(sinT)


def _rot_matrix():
    # rot(q)[d] = -q[d+64] (d<64) ; q[d-64] (d>=64);  rot = P @ q (q as [hd] col)
    P = np.zeros((HD, HD), dtype=np.float32)
    half = HD // 2
    for d in range(half):
        P[d, d + half] = -1.0
        P[d + half, d] = 1.0
    return np.ascontiguousarray(P.T.astype(NP_BF16))  # lhsT for out = P @ rhs


def make_in_maps(x, Wq, Wk, Wv, Wo, past_k, past_v):
    xt = np.ascontiguousarray(x.reshape(TOK, H).T.astype(NP_BF16))
    cosT, sinT = _rope_tables()
    prot = _rot_matrix()
    in_maps = []
    for c in range(N_CORES):
        h0 = c * NHL
        cols = slice(h0 * HD, (h0 + NHL) * HD)
        # K slice pre-transposed to [nhl, B, HD, KV]
        ktc = np.ascontiguousarray(
            past_k[:, h0:h0 + NHL].astype(NP_BF16).transpose(1, 0, 3, 2))
        # V slice chunk-transposed to [nhl, B, 128, NCH, 128]:
        # vtc[h, b, p, c, d] = past_v[b, h0+h, c*128+p, d]
        vtc = np.ascontiguousarray(
            past_v[:, h0:h0 + NHL].astype(NP_BF16)
            .reshape(B, NHL, NCH, 128, HD).transpose(1, 0, 3, 2, 4))
        in_maps.append({
            "kt": ktc,
            "vc": vtc,
            "xt": xt,
            "wq": np.ascontiguousarray(Wq[:, cols].astype(NP_BF16)),
            "wk": np.ascontiguousarray(Wk[:, cols].astype(NP_BF16)),
            "wv": np.ascontiguousarray(Wv[:, cols].astype(NP_BF16)),
            "wo": np.ascontiguousarray(Wo[cols, :].astype(NP_BF16)),
            "cost": cosT,
            "sint": sinT,
            "prot": prot,
        })
    return in_maps


def kernel(x, Wq, Wk, Wv, Wo, past_k, past_v):
    x = np.asarray(x, dtype=np.float32)
    Wq = np.asarray(Wq, dtype=np.float32)
    Wk = np.asarray(Wk, dtype=np.float32)
    Wv = np.asarray(Wv, dtype=np.float32)
    Wo = np.asarray(Wo, dtype=np.float32)
    past_k = np.asarray(past_k, dtype=np.float32)
    past_v = np.asarray(past_v, dtype=np.float32)

    nc = get_nc()
    in_maps = make_in_maps(x, Wq, Wk, Wv, Wo, past_k, past_v)
    res = run_bass_kernel_spmd(nc, in_maps, core_ids=list(range(N_CORES)))
    y = res.results[0]["out"]
    return np.asarray(y, dtype=np.float32).reshape(B, S, H)


# revision 9
# speedup vs baseline: 3.6025x; 1.1394x over previous
"""Trainium2 Bass kernel for multi-head attention decode (B=16, S=8, H=2048,
16 heads x 128 head_dim, KV cache 4096) sharded over 8 NeuronCores by heads
(tensor parallel, 2 heads/core), with chunked on-device AllReduce after
o_proj overlapped under the attention loop.

Compute/storage dtype is bf16 (KV cache, weights, activations; all matmul
accumulation in fp32 PSUM) which halves HBM traffic vs f32 and enables the
PE fast-weight-load path. The K cache is stored pre-transposed [hd, kv] and
the V cache chunk-transposed [kv%128, kv//128, hd] on host so both stream
as fully contiguous 8KB-per-partition DMAs.

Self-contained: hardcodes all shapes/sharding. Host side only reshapes /
slices / casts the sharded inputs and gathers the full output.
"""

import numpy as np
import ml_dtypes

import concourse.bass as bass
import concourse.tile as tile
import concourse.mybir as mybir
from concourse import bacc
from concourse.bass_utils import run_bass_kernel_spmd

F32 = mybir.dt.float32
BF16 = mybir.dt.bfloat16
NP_BF16 = ml_dtypes.bfloat16

N_CORES = 8
B = 16
S = 8
H = 2048
NH = 16           # total heads
HD = 128          # head dim
KV = 4096         # past kv length
NHL = NH // N_CORES   # heads per core = 2
TOK = B * S       # 128 tokens
NCH = KV // 128   # 32 kv chunks of 128 per (h,b)
SCALE = 1.0 / float(np.sqrt(HD))

N_CHUNKS = 4              # output chunks for overlapped AllReduce
CB = B // N_CHUNKS        # batches per chunk = 4
CTOK = CB * S             # tokens per chunk = 32

_CACHED_NC = None


def _build_nc(with_collective=True, kv_bufs=4):
    nc = bacc.Bacc(
        "TRN2",
        target_bir_lowering=False,
        debug=False,
        enable_asserts=False,
        num_devices=N_CORES if with_collective else 1,
    )

    kt = nc.dram_tensor("kt", [NHL, B, HD, KV], BF16, kind="ExternalInput")
    vc = nc.dram_tensor("vc", [NHL, B, 128, NCH, 128], BF16, kind="ExternalInput")
    xt = nc.dram_tensor("xt", [H, TOK], BF16, kind="ExternalInput")
    wq = nc.dram_tensor("wq", [H, NHL * HD], BF16, kind="ExternalInput")
    wk = nc.dram_tensor("wk", [H, NHL * HD], BF16, kind="ExternalInput")
    wv = nc.dram_tensor("wv", [H, NHL * HD], BF16, kind="ExternalInput")
    wo = nc.dram_tensor("wo", [NHL * HD, H], BF16, kind="ExternalInput")
    cost = nc.dram_tensor("cost", [HD, TOK], BF16, kind="ExternalInput")
    sint = nc.dram_tensor("sint", [HD, TOK], BF16, kind="ExternalInput")
    prot = nc.dram_tensor("prot", [HD, HD], BF16, kind="ExternalInput")
    out = nc.dram_tensor("out", [TOK, H], F32, kind="ExternalOutput")

    KT16 = H // 128  # 16 contraction tiles over H

    with tile.TileContext(nc) as tc:
        with tc.tile_pool(name="const", bufs=1) as const, \
             tc.tile_pool(name="kv_io", bufs=kv_bufs) as kvp:
            # Resident weights / activations. Queue order matters: these
            # sit ahead of the kv-cache stream on the sync/scalar HWDGE
            # queues, so keep early-needed tensors first.
            xt_sb = const.tile([128, KT16, TOK], BF16)
            nc.sync.dma_start(out=xt_sb, in_=xt.ap().rearrange("(t p) n -> p t n", p=128))
            wq_sb = const.tile([128, KT16, NHL * HD], BF16)
            nc.sync.dma_start(out=wq_sb, in_=wq.ap().rearrange("(t p) m -> p t m", p=128))
            wk_sb = const.tile([128, KT16, NHL * HD], BF16)
            nc.sync.dma_start(out=wk_sb, in_=wk.ap().rearrange("(t p) m -> p t m", p=128))
            cost_sb = const.tile([HD, TOK], BF16)
            nc.scalar.dma_start(out=cost_sb, in_=cost.ap())
            sint_sb = const.tile([HD, TOK], BF16)
            nc.scalar.dma_start(out=sint_sb, in_=sint.ap())
            prot_sb = const.tile([HD, HD], BF16)
            nc.scalar.dma_start(out=prot_sb, in_=prot.ap())
            wv_sb = const.tile([128, KT16, NHL * HD], BF16)
            nc.scalar.dma_start(out=wv_sb, in_=wv.ap().rearrange("(t p) m -> p t m", p=128))
            ones_sb = const.tile([128, 1], BF16)
            nc.vector.memset(ones_sb, 1.0)
            onesrow_sb = const.tile([1, TOK], F32)
            nc.vector.memset(onesrow_sb, 1.0)

            qT = [const.tile([HD, TOK], BF16, name=f"qT{h}") for h in range(NHL)]
            kT = [const.tile([HD, TOK], BF16, name=f"kT{h}") for h in range(NHL)]
            vstage = [const.tile([S, B, HD], BF16, name=f"vstage{h}") for h in range(NHL)]
            # unnormalized attention out (transposed) + per-token softmax denoms
            oTu_sb = [const.tile([HD, TOK], F32, name=f"oTu{h}") for h in range(NHL)]
            rsh_sb = [const.tile([1, TOK], F32, name=f"rsh{h}") for h in range(NHL)]
            oT_sb = [const.tile([HD, TOK], BF16, name=f"oT{h}") for h in range(NHL)]

            # batch-major pair order so a chunk of CB batches completes
            # (both heads) before its o_proj + AllReduce chunk
            hb = [(h, b) for b in range(B) for h in range(NHL)]

            prefetched = {}

            def prefetch(i):
                if i >= len(hb):
                    return
                h, b = hb[i]
                kt_t = kvp.tile([128, KV], BF16, tag="kt", name="kt_t")
                v_t = kvp.tile([128, NCH, 128], BF16, tag="vt", name="v_t")
                nc.sync.dma_start(out=kt_t, in_=kt.ap()[h, b])
                nc.scalar.dma_start(out=v_t, in_=vc.ap()[h, b])
                prefetched[i] = (kt_t, v_t)

            # start the kv-cache stream before the projections compute
            prefetch(0)
            prefetch(1)

            # wo is needed only at the first chunk epilogue (~60us in)
            wo_sb = const.tile([128, NHL, H], BF16)
            nc.scalar.dma_start(out=wo_sb, in_=wo.ap().rearrange("(t p) n -> p t n", p=128))

            # ---- Phase 1: projections + RoPE (all in [hd, tok] layout) ----
            with tc.tile_pool(name="proj_ps", bufs=2, space="PSUM") as pps, \
                 tc.tile_pool(name="proj_tmp", bufs=2) as ptp, \
                 tc.tile_pool(name="vns_dram", bufs=1, space="DRAM") as vnd:
                for h in range(NHL):
                    for w_sb, dst in ((wq_sb, qT[h]), (wk_sb, kT[h])):
                        ps = pps.tile([128, 128], F32, tag="projps", name="ps")
                        for t in range(KT16):
                            nc.tensor.matmul(
                                ps,
                                lhsT=w_sb[:, t, h * HD:(h + 1) * HD],
                                rhs=xt_sb[:, t, :],
                                start=(t == 0),
                                stop=(t == KT16 - 1),
                            )
                        raw = ptp.tile([128, 128], BF16, tag="raw", name="raw")
                        nc.vector.tensor_copy(out=raw, in_=ps)
                        rot_ps = pps.tile([128, 128], F32, tag="projps", name="rot_ps")
                        nc.tensor.matmul(rot_ps, lhsT=prot_sb, rhs=raw,
                                         start=True, stop=True)
                        tmp = ptp.tile([128, 128], F32, tag="tmp", name="tmp")
                        nc.vector.tensor_mul(out=tmp, in0=raw, in1=cost_sb)
                        tmp2 = ptp.tile([128, 128], F32, tag="tmp2", name="tmp2")
                        nc.vector.tensor_mul(out=tmp2, in0=rot_ps, in1=sint_sb)
                        nc.vector.tensor_add(out=dst, in0=tmp2, in1=tmp)

                # v_new = x @ Wv  -> [tok, 2*128] (natural layout)
                ps_v = pps.tile([128, NHL * HD], F32, tag="projps", name="ps_v")
                for t in range(KT16):
                    nc.tensor.matmul(ps_v, lhsT=xt_sb[:, t, :], rhs=wv_sb[:, t, :],
                                     start=(t == 0), stop=(t == KT16 - 1))
                vnew_sb = ptp.tile([128, NHL * HD], BF16, tag="vnew", name="vnew_sb")
                nc.vector.tensor_copy(out=vnew_sb, in_=ps_v)
                # restage at partition base 0 as [s, b, hd], via a DRAM
                # bounce (a partition-crossing gather is cheap on the DRAM
                # side; 32 tiny per-(h,b) SBUF-to-SBUF SWDGE DMAs took ~1us
                # fixed cost each and stalled the whole pipeline)
                vns = vnd.tile([TOK, NHL * HD], BF16, name="vns")
                nc.sync.dma_start(out=vns, in_=vnew_sb)
                for h in range(NHL):
                    nc.scalar.dma_start(
                        out=vstage[h],
                        in_=vns[:, h * HD:(h + 1) * HD]
                            .rearrange("(b s) d -> s b d", s=S),
                    )

            # ---- Phase 2+3+4 interleaved: attention over the KV cache,
            # with per-chunk normalize + o_proj + AllReduce epilogues ----
            with tc.tile_pool(name="esb", bufs=2) as etp, \
                 tc.tile_pool(name="nrm", bufs=2) as nrm, \
                 tc.tile_pool(name="ysb", bufs=2) as yp, \
                 tc.tile_pool(name="ps_s", bufs=2, space="PSUM") as psp, \
                 tc.tile_pool(name="ps_o", bufs=1, space="PSUM") as pso, \
                 tc.tile_pool(name="ps_rs", bufs=1, space="PSUM") as psr, \
                 tc.tile_pool(name="ps_epi", bufs=1, space="PSUM") as pse, \
                 tc.tile_pool(name="dram", bufs=1, space="DRAM") as dram:

                stage = {}  # pipelined state for step i

                def emit_scores(i):
                    h, b = hb[i]
                    qcol = qT[h][:, b * S:(b + 1) * S]
                    kt_t, v_t = prefetched.pop(i)
                    # cols 0..255: past-kv scores; cols 256..263: new-token scores
                    s_ps = psp.tile([128, (NCH + 1) * S], F32, tag="sps", name="s_ps")
                    for c in range(NCH):
                        nc.tensor.matmul(
                            s_ps[:, c * S:(c + 1) * S],
                            lhsT=kt_t[:, c * 128:(c + 1) * 128],
                            rhs=qcol,
                            start=True, stop=True)
                    nc.tensor.matmul(
                        s_ps[0:S, NCH * S:(NCH + 1) * S],
                        lhsT=kT[h][:, b * S:(b + 1) * S],
                        rhs=qcol, start=True, stop=True)
                    eT = etp.tile([128, (NCH + 1) * S], BF16, tag="eT", name="eT")
                    nc.scalar.activation(out=eT[:, 0:NCH * S], in_=s_ps[:, 0:NCH * S],
                                         func=mybir.ActivationFunctionType.Exp,
                                         scale=SCALE)
                    nc.scalar.activation(out=eT[0:S, NCH * S:(NCH + 1) * S],
                                         in_=s_ps[0:S, NCH * S:(NCH + 1) * S],
                                         func=mybir.ActivationFunctionType.Exp,
                                         scale=SCALE)
                    stage[i] = (eT, v_t)

                def emit_attnv(i):
                    h, b = hb[i]
                    eT, v_t = stage.pop(i)
                    eTn = eT[0:S, NCH * S:(NCH + 1) * S]
                    # oT2_ps[d, slot, s]: rotating accumulators in SEPARATE
                    # PSUM banks (bank = 512 f32) so consecutive matmuls never
                    # RMW the same accumulation address (drain pipelining)
                    NSLOT = 2
                    oT2_ps = pso.tile([HD, NSLOT, 512], F32, tag="ops", name="oT2_ps")
                    for c in range(NCH):
                        nc.tensor.matmul(
                            oT2_ps[:, c % NSLOT, 0:S],
                            lhsT=v_t[:, c, :],
                            rhs=eT[:, c * S:(c + 1) * S],
                            start=(c < NSLOT),
                            stop=(c >= NCH - NSLOT + 1))
                    # new tokens (kv positions 4096..4103) -> slot 0, last
                    nc.tensor.matmul(oT2_ps[:, 0, 0:S], lhsT=vstage[h][:, b, :],
                                     rhs=eTn, start=False, stop=True)
                    # rowsums: ones^T @ eT -> [1, (c s)] partials in one matmul
                    rs_ps = psr.tile([1, (NCH + 1) * S], F32, tag="rsps",
                                     name="rs_ps")
                    nc.tensor.matmul(rs_ps[:, 0:NCH * S], lhsT=ones_sb,
                                     rhs=eT[:, 0:NCH * S],
                                     start=True, stop=False)
                    nc.tensor.matmul(rs_ps[:, NCH * S:(NCH + 1) * S],
                                     lhsT=ones_sb[0:S, :],
                                     rhs=eTn, start=False, stop=True)
                    # evacuate: fold the slots -> unnormalized oT column block
                    nc.vector.reduce_sum(
                        out=oTu_sb[h][:, b * S:(b + 1) * S],
                        in_=oT2_ps[:, :, 0:S].rearrange("p g s -> p s g"),
                        axis=mybir.AxisListType.X)
                    nc.vector.reduce_sum(
                        out=rsh_sb[h][:, b * S:(b + 1) * S],
                        in_=rs_ps.rearrange("p (c s) -> p s c", s=S),
                        axis=mybir.AxisListType.X)

                def emit_epilogue(cidx):
                    cols = slice(cidx * CTOK, (cidx + 1) * CTOK)
                    # normalize: oT = oTu * (1/rs) broadcast down partitions
                    for h in range(NHL):
                        recip = nrm.tile([1, CTOK], F32, tag="recip", name="recip")
                        nc.vector.reciprocal(out=recip, in_=rsh_sb[h][:, cols])
                        bc_ps = psr.tile([HD, CTOK], F32, tag="bc", name="bc_ps")
                        nc.tensor.matmul(bc_ps, lhsT=onesrow_sb[:, 0:HD], rhs=recip,
                                         start=True, stop=True)
                        nc.vector.tensor_mul(out=oT_sb[h][:, cols],
                                             in0=oTu_sb[h][:, cols], in1=bc_ps)
                    # o_proj for this chunk's CTOK tokens
                    y_c = yp.tile([CTOK, H], F32, tag="yc", name="y_c")
                    for nb in range(H // 512):
                        y_ps = pse.tile([CTOK, 512], F32, tag="yps", name="y_ps")
                        for h in range(NHL):
                            nc.tensor.matmul(
                                y_ps,
                                lhsT=oT_sb[h][:, cols],
                                rhs=wo_sb[:, h, nb * 512:(nb + 1) * 512],
                                start=(h == 0), stop=(h == NHL - 1))
                        nc.vector.tensor_copy(out=y_c[:, nb * 512:(nb + 1) * 512],
                                              in_=y_ps)
                    if with_collective:
                        y_in = dram.tile([CTOK, H], F32, name=f"y_in{cidx}")
                        nc.sync.dma_start(out=y_in, in_=y_c)
                        y_out = dram.tile([CTOK, H], F32, addr_space="Shared",
                                          name=f"y_out{cidx}")
                        nc.gpsimd.collective_compute(
                            "AllReduce",
                            mybir.AluOpType.add,
                            replica_groups=[list(range(N_CORES))],
                            ins=[y_in[:]],
                            outs=[y_out[:]],
                        )
                        nc.sync.dma_start(out=out.ap()[cols], in_=y_out)
                    else:
                        nc.sync.dma_start(out=out.ap()[cols], in_=y_c)

                PPC = NHL * CB  # pairs per chunk = 8
                emit_scores(0)
                for j in range(2, kv_bufs):
                    prefetch(j)
                for i in range(1, len(hb)):
                    emit_scores(i)
                    emit_attnv(i - 1)
                    prefetch(i + kv_bufs - 1)
                    if i % PPC == 0:
                        emit_epilogue(i // PPC - 1)
                emit_attnv(len(hb) - 1)
                emit_epilogue(N_CHUNKS - 1)

    nc.compile()
    return nc


def get_nc():
    global _CACHED_NC
    if _CACHED_NC is None:
        _CACHED_NC = _build_nc()
    return _CACHED_NC


def _rope_tables():
    inv_freq = (1.0 / (10000.0 ** (np.arange(0, HD, 2, dtype=np.float32) / HD))).astype(np.float32)
    t = np.arange(S, dtype=np.float32)
    freqs = t[:, None] * inv_freq[None, :]          # [S, HD/2]
    emb = np.concatenate([freqs, freqs], axis=-1)   # [S, HD]
    cos = np.cos(emb).astype(np.float32)            # [S, HD]
    sin = np.sin(emb).astype(np.float32)
    # transposed+tiled over batches: [HD, B*S] with col b*S+s = table row s
    cosT = np.tile(cos.T, (1, B)).astype(NP_BF16)
    sinT = np.tile(sin.T, (1, B)).astype(NP_BF16)
    return np.ascontiguousarray(cosT), np.ascontiguousarray(sinT)


def _rot_matrix():
    # rot(q)[d] = -q[d+64] (d<64) ; q[d-64] (d>=64);  rot = P @ q (q as [hd] col)
    P = np.zeros((HD, HD), dtype=np.float32)
    half = HD // 2
    for d in range(half):
        P[d, d + half] = -1.0
        P[d + half, d] = 1.0
    return np.ascontiguousarray(P.T.astype(NP_BF16))  # lhsT for out = P @ rhs


def make_in_maps(x, Wq, Wk, Wv, Wo, past_k, past_v):
    xt = np.ascontiguousarray(x.reshape(TOK, H).T.astype(NP_BF16))
    cosT, sinT = _rope_tables()
    prot = _rot_matrix()
    in_maps = []
    for c in range(N_CORES):
        h0 = c * NHL
        cols = slice(h0 * HD, (h0 + NHL) * HD)
        # K slice pre-transposed to [nhl, B, HD, KV]
        ktc = np.ascontiguousarray(
            past_k[:, h0:h0 + NHL].astype(NP_BF16).transpose(1, 0, 3, 2))
        # V slice chunk-transposed to [nhl, B, 128, NCH, 128]:
        # vtc[h, b, p, c, d] = past_v[b, h0+h, c*128+p, d]
        vtc = np.ascontiguousarray(
            past_v[:, h0:h0 + NHL].astype(NP_BF16)
            .reshape(B, NHL, NCH, 128, HD).transpose(1, 0, 3, 2, 4))
        in_maps.append({
            "kt": ktc,
            "vc": vtc,
            "xt": xt,
            "wq": np.ascontiguousarray(Wq[:, cols].astype(NP_BF16)),
            "wk": np.ascontiguousarray(Wk[:, cols].astype(NP_BF16)),
            "wv": np.ascontiguousarray(Wv[:, cols].astype(NP_BF16)),
            "wo": np.ascontiguousarray(Wo[cols, :].astype(NP_BF16)),
            "cost": cosT,
            "sint": sinT,
            "prot": prot,
        })
    return in_maps


def kernel(x, Wq, Wk, Wv, Wo, past_k, past_v):
    x = np.asarray(x, dtype=np.float32)
    Wq = np.asarray(Wq, dtype=np.float32)
    Wk = np.asarray(Wk, dtype=np.float32)
    Wv = np.asarray(Wv, dtype=np.float32)
    Wo = np.asarray(Wo, dtype=np.float32)
    past_k = np.asarray(past_k, dtype=np.float32)
    past_v = np.asarray(past_v, dtype=np.float32)

    nc = get_nc()
    in_maps = make_in_maps(x, Wq, Wk, Wv, Wo, past_k, past_v)
    res = run_bass_kernel_spmd(nc, in_maps, core_ids=list(range(N_CORES)))
    y = res.results[0]["out"]
    return np.asarray(y, dtype=np.float32).reshape(B, S, H)
